# revision 19
# baseline (speedup 1.0000x reference)
"""Multi-head attention (B=4, S=2048, D=1024, H=16, causal) on 8 trn2 cores.

Sharding: core = b*2 + g  (b = batch 0..3, g = head-group 0..1, 8 heads each).
All matmul operands are bf16 (enables Fast Weight Load on LDWEIGHTS and halves
input DMA); PSUM accumulation stays fp32.

The kernel is emitted as an interleaved software pipeline over 512-wide
sequence chunks so the Tile scheduler can overlap everything:

    A(0) B(0) A(1) C(0) B(1) A(2) C(1) B(2) A(3) C(2) B(3) C(3)

  A(s): Q^T/K^T projections for chunk s (d-major, per-chunk tiles
        QTc/KTc[db][s] = [128 d, 512 s]) and V projection for the four
        128-row key blocks of chunk s (s-major VT[kb] = [128 s, 8 h, 64+1]
        with an appended ones column per head -- the AV matmul then emits
        softmax denominators in PSUM row 64 for free).
  B(qb): for each head pair hp: transposed scores S^T = K_h Q_h^T per 128-key
        block, both heads row-tiled into disjoint PE row groups (concurrent)
        into one [128,2,512] PSUM tile; one Exp ACTIVATE per block covers
        both heads; causal masking = DVE multiply of the diagonal block's
        exp'd probabilities by a 0/1 lower-triangle tile (no PE mask matmul);
        O^T accumulated over key blocks into a [128,2,512] PSUM pair drawn
        from the same 3-slot pool (so the next pair's AV can start while the
        previous normalizes); normalization = DVE reciprocal of the PSUM
        denominator row + GpSimd partition-broadcast + DVE multiply into
        OTc[hp][qb] (bf16).  No PE instructions in the normalization path.
  C(qb): out[qb chunk] = OTc^T @ Wo, bias added by DVE during the PSUM->SBUF
        copy (pre-broadcast [128,D] bias tile), DMA to HBM.

Other details: ~60 warmup matmuls on a scratch tile at kernel start keep the
PE HAM clock-gate warm while the initial DMAs land; the first chunk's
weight/activation DMAs are split per contraction tile across the 16 DMA
queues so the first projection starts as early as possible.

Host: input transpose/bf16-cast + shard; the g-pair partial sum (row-parallel
Wo all-reduce) happens at gather time.
"""

import numpy as np

S = 2048
D = 1024
DL = 512          # local head dims per core (8 heads x 64)
HL = 8            # local heads
DK = 64
NBK = D // 128    # contraction tiles for projections
NDB = DL // 128   # d-out blocks (head pairs)
NQ = S // 512     # q blocks
NBS = S // 128    # s tiles / key blocks

_NC = {}


def _build_nc():
    import concourse.bass as bass
    import concourse.mybir as mybir
    import concourse.tile as tile
    from concourse import bacc

    F32 = mybir.dt.float32
    F32R = mybir.dt.float32r
    BF16 = mybir.dt.bfloat16
    Exp = mybir.ActivationFunctionType.Exp

    nc = bacc.Bacc(None)

    xq = nc.dram_tensor("xq", [128, NBK, S], BF16, kind="ExternalInput")
    xk = nc.dram_tensor("xk", [128, NBK, S], BF16, kind="ExternalInput")
    xv = nc.dram_tensor("xv", [128, NBK, S], BF16, kind="ExternalInput")
    wq = nc.dram_tensor("wq", [128, NBK, DL], BF16, kind="ExternalInput")
    wk = nc.dram_tensor("wk", [128, NBK, DL], BF16, kind="ExternalInput")
    wv = nc.dram_tensor("wv", [128, NBK, DL], BF16, kind="ExternalInput")
    wo = nc.dram_tensor("wo", [128, NDB, D], BF16, kind="ExternalInput")
    bqs = nc.dram_tensor("bqs", [128, NDB], F32, kind="ExternalInput")
    bks = nc.dram_tensor("bks", [128, NDB], F32, kind="ExternalInput")
    bvb = nc.dram_tensor("bvb", [128, DL], BF16, kind="ExternalInput")
    bob = nc.dram_tensor("bob", [128, D], BF16, kind="ExternalInput")
    trib = nc.dram_tensor("trib", [128, 2, 128], BF16, kind="ExternalInput")
    onesr = nc.dram_tensor("onesr", [1, DK], F32R, kind="ExternalInput")
    out_d = nc.dram_tensor("out", [S, D], F32, kind="ExternalOutput")

    with tile.TileContext(nc) as tc, nc.allow_low_precision(
            reason="bf16 matmul operands are intended"):
        with (
            tc.tile_pool(name="const", bufs=1) as cpool,
            tc.tile_pool(name="res", bufs=1) as rpool,
            tc.tile_pool(name="xt", bufs=3) as xpool,
            tc.tile_pool(name="pt", bufs=18) as ptpool,
            tc.tile_pool(name="rc", bufs=2) as rcpool,
            tc.tile_pool(name="rb", bufs=2) as rbpool,
            tc.tile_pool(name="ot", bufs=3) as otpool,
            tc.tile_pool(name="mp", bufs=2, space="PSUM") as mpool,
            tc.tile_pool(name="bp", bufs=3, space="PSUM") as bpool,
        ):
            # PE warmup: the HAM clock gate needs ~3.4us of sustained matmul
            # activity to lift the PE clock from 1.2 to 2.4 GHz.  Spend the
            # initial DMA wait running matmuls on a scratch tile so the real
            # projections start warm.
            warm_sb = cpool.tile([128, 128], BF16, name="warm", tag="warm")
            nc.vector.memset(warm_sb[:], 0.0)
            warm_ps = mpool.tile([128, 512], F32, name="mp", tag="mp")
            for _ in range(40):
                nc.tensor.matmul(warm_ps[:, 0:128], warm_sb[:], warm_sb[:],
                                 start=True, stop=True)

            # Startup order matters: the first projection only needs
            # bqs/bks + wq + the first xq chunk, so issue those DMAs first
            # (split per contraction tile so they spread across the 16 DMA
            # queues) and defer the B/C-phase constants until after the
            # first A chunk is emitted.
            bqs_sb = cpool.tile([128, NDB], F32, name="bqs", tag="bqs")
            bks_sb = cpool.tile([128, NDB], F32, name="bks", tag="bks")
            bvb_sb = cpool.tile([128, DL], BF16, name="bvb", tag="bvb")
            bob_sb = cpool.tile([128, D], BF16, name="bob", tag="bob")
            trib_sb = cpool.tile([128, 2, 128], BF16, name="trib", tag="trib")
            onesr_sb = cpool.tile([1, DK], F32R, name="onesr", tag="onesr")
            nc.sync.dma_start(bqs_sb[:], bqs[:])
            nc.sync.dma_start(bks_sb[:], bks[:])

            wq_sb = rpool.tile([128, NBK, DL], BF16, name="wq", tag="wq")
            wk_sb = rpool.tile([128, NBK, DL], BF16, name="wk", tag="wk")
            wv_sb = rpool.tile([128, NBK, DL], BF16, name="wv", tag="wv")
            wo_sb = rpool.tile([128, NDB, D], BF16, name="wo", tag="wo")

            QTc = [[rpool.tile([128, 512], BF16, name=f"QT{i}_{s}", tag=f"QT{i}_{s}")
                    for s in range(NQ)] for i in range(NDB)]
            KTc = [[rpool.tile([128, 512], BF16, name=f"KT{i}_{s}", tag=f"KT{i}_{s}")
                    for s in range(NQ)] for i in range(NDB)]
            # VT is flat [128, 520] = 8 heads x (64 V dims + a ones column);
            # the AV stationary operand is head h's 65-column window, whose
            # ones column makes PSUM row 64 the softmax denominator for free.
            # (65-col LDWEIGHTS gets no Fast Weight Load, but at 65 columns
            # it costs the same as a 128-col FWL load.)
            VT = [rpool.tile([128, HL * (DK + 1)], BF16,
                             name=f"VT{i}", tag=f"VT{i}")
                  for i in range(NBS)]
            OTc = [[rpool.tile([128, 512], BF16, name=f"OT{i}_{s}", tag=f"OT{i}_{s}")
                    for s in range(NQ)] for i in range(NDB)]

            def phase_a_dma(s, first=False):
                sl = slice(s * 512, (s + 1) * 512)
                xts = []
                for xd, w_sb_, wd in ((xq, wq_sb, wq), (xk, wk_sb, wk),
                                      (xv, wv_sb, wv)):
                    xt = xpool.tile([128, NBK, 512], BF16, name="xt", tag="xt")
                    if first:
                        # Interleave per-k weight and activation pieces so the
                        # k-loop of the first projection can start after the
                        # first pair lands and chase the rest.
                        for k in range(NBK):
                            nc.sync.dma_start(w_sb_[:, k, :], wd[:, k, :])
                            nc.sync.dma_start(xt[:, k, :], xd[:, k, sl])
                    else:
                        nc.sync.dma_start(xt[:], xd[:, :, sl])
                    xts.append(xt)
                if first:
                    nc.sync.dma_start(bvb_sb[:], bvb[:])
                return xts

            def phase_a_compute(s, xts):
                # Head-pair 0 of Q/K and all of V before head-pairs 1..3, so
                # B(qb=s, hp=0) unblocks early.
                def proj_db(xt, w_sb_, b_sb, dst, db):
                    ps = mpool.tile([128, 512], F32, name="mp", tag="mp")
                    for k in range(NBK):
                        nc.tensor.matmul(
                            ps[:], w_sb_[:, k, db * 128:(db + 1) * 128],
                            xt[:, k, :], start=(k == 0), stop=(k == NBK - 1))
                    nc.vector.tensor_scalar_add(
                        dst[db][s][:], ps[:], b_sb[:, db:db + 1])

                proj_db(xts[0], wq_sb, bqs_sb, QTc, 0)
                proj_db(xts[1], wk_sb, bks_sb, KTc, 0)
                for mi in range(4):
                    m = 4 * s + mi
                    ps = mpool.tile([128, 512], F32, name="mp", tag="mp")
                    for k in range(NBK):
                        nc.tensor.matmul(
                            ps[:], xts[2][:, k, mi * 128:(mi + 1) * 128],
                            wv_sb[:, k, :], start=(k == 0), stop=(k == NBK - 1))
                    vt3 = VT[m][:, 0:HL * (DK + 1)].rearrange(
                        "p (h c) -> p h c", c=DK + 1)
                    nc.vector.memset(vt3[:, :, DK:DK + 1], 1.0)
                    nc.vector.tensor_add(
                        vt3[:, :, 0:DK],
                        ps[:].rearrange("p (h c) -> p h c", c=DK),
                        bvb_sb[:].rearrange("p (h c) -> p h c", c=DK))
                for db in range(1, NDB):
                    proj_db(xts[0], wq_sb, bqs_sb, QTc, db)
                    proj_db(xts[1], wk_sb, bks_sb, KTc, db)

            # B runs as a software pipeline over (qb, hp) iterations: while
            # iteration i computes scores+exp (ACT-bound), the AV chain of
            # iteration i-1 -- whose pt tiles are all ready -- streams
            # back-to-back on the PE with no dependency waits.
            bstate = {"pending": [], "norm2": None, "norm2_age": 0}

            # The norm is split in two: stage 1 (DVE reciprocal of the PSUM
            # denominator row + GpSimd broadcast across the 64 d partitions)
            # fires as soon as an AV chain completes; stage 2 (the DVE
            # multiplies that read pso and free its PSUM slot) is deferred a
            # couple of key blocks so the GpSimd broadcast latency is hidden.
            def emit_norm1(it):
                # reciprocal_approx_fast is a bitwise custom DVE op -- it
                # only reads correctly from SBUF, so stage the PSUM
                # denominator row through an SBUF copy first.
                den = rcpool.tile([1, 2, 512], F32, name="rc", tag="rc")
                nc.vector.tensor_copy(den[:], it["pso"][DK:DK + 1, :, :])
                it["rden"] = rcpool.tile([1, 2, 512], F32, name="rc2", tag="rc2")
                nc.vector.reciprocal_approx_fast(
                    out=it["rden"][:], in_=den[:])
                it["rb"] = rbpool.tile([DK, 2, 512], F32, name="rb", tag="rb")
                nc.gpsimd.partition_broadcast(
                    it["rb"][:], it["rden"][:], channels=DK)

            def emit_norm1_fast(it):
                # Tail fast path (last iteration): the GpSimd broadcast's
                # ~1.8us latency would sit serially in front of phase_c(3),
                # so broadcast on the (by then idle) PE instead, like the
                # denominator row itself: stationary ones [1, DK], moving
                # f32r denominator row.
                den = rcpool.tile([1, 2, 512], F32R, name="rcf", tag="rcf")
                nc.vector.tensor_copy(den[:], it["pso"][DK:DK + 1, :, :])
                rb = rbpool.tile([DK, 2, 512], F32, name="rbf", tag="rbf")
                for h2 in range(2):
                    psb = mpool.tile([DK, 512], F32, name="mp", tag="mp")
                    nc.tensor.matmul(psb[:], onesr_sb[0:1, :],
                                     den[0:1, h2, :], start=True, stop=True,
                                     skip_group_check=True)
                    nc.vector.reciprocal_approx_fast(
                        out=rb[:, h2, :], in_=psb[:])
                it["rb"] = rb

            def emit_norm2(it, split=False):
                hp, qb, pso, rb = it["hp"], it["qb"], it["pso"], it["rb"]
                if split:
                    # Column-split so phase_c's mi blocks unblock ASAP.
                    for q4 in range(4):
                        ql = slice(q4 * 128, (q4 + 1) * 128)
                        for h2 in range(2):
                            nc.vector.tensor_mul(
                                OTc[hp][qb][h2 * DK:(h2 + 1) * DK, ql],
                                pso[0:DK, h2, ql], rb[:, h2, ql])
                    return
                for h2 in range(2):
                    nc.vector.tensor_mul(
                        OTc[hp][qb][h2 * DK:(h2 + 1) * DK, :],
                        pso[0:DK, h2, :], rb[:, h2, :])

            def flush_norm2():
                it = bstate.get("norm2")
                if it is not None:
                    emit_norm2(it)
                    bstate["norm2"] = None

            def emit_avs(it, n):
                """Emit AV matmuls for iteration `it` up to index n; when the
                chain completes, normalize immediately so the pso slot frees
                as early as possible."""
                if it is None or it.get("done"):
                    return
                while it["emitted"] < min(n, it["kbmax"]):
                    kb, pt_, minq = it["pts"][it["emitted"]]
                    if it["pso"] is None:
                        it["pso"] = bpool.tile([128, 2, 512], F32,
                                               name="bp", tag="bp")
                    for h2 in range(2):
                        h = it["hp"] * 2 + h2
                        nc.tensor.matmul(
                            it["pso"][0:DK + 1, h2, minq:512],
                            VT[kb][:, h * (DK + 1):(h + 1) * (DK + 1)],
                            pt_[:, h2, minq:512],
                            start=(kb == 0), stop=(kb == it["kbmax"] - 1),
                            skip_group_check=True)
                    it["emitted"] += 1
                if it["emitted"] == it["kbmax"]:
                    it["done"] = True
                    if it.get("fast"):
                        emit_norm1_fast(it)
                        emit_norm2(it, split=True)
                    else:
                        emit_norm1(it)
                        flush_norm2()
                        bstate["norm2"] = it
                        bstate["norm2_age"] = 0

            def b_iter(qb, hp, lag=1, pace_delay=0, last=False):
                kbmax = 4 * (qb + 1)
                cur = {"hp": hp, "qb": qb, "kbmax": kbmax, "fast": last,
                       "pts": [], "pso": None, "emitted": 0}
                for kb in range(kbmax):
                    # Ready AV matmuls go in front of the score matmul: the
                    # score may wait on a PSUM slot (exp pacing) and the PE
                    # queue is strict FIFO.
                    if bstate["pending"]:
                        head = bstate["pending"][0]
                        emit_avs(head, kb + 2 - pace_delay)
                        if head.get("done"):
                            bstate["pending"].pop(0)
                    if bstate.get("norm2") is not None:
                        bstate["norm2_age"] += 1
                        if bstate["norm2_age"] >= 2:
                            flush_norm2()
                    di = kb - 4 * qb
                    minq = 128 * di if di > 0 else 0
                    pss = bpool.tile([128, 2, 512], F32, name="bp", tag="bp")
                    for h2 in range(2):
                        base = h2 * DK
                        nc.tensor.matmul(
                            pss[:, h2, minq:512],
                            KTc[hp][kb // 4][base:base + DK,
                                             (kb % 4) * 128:(kb % 4 + 1) * 128],
                            QTc[hp][qb][base:base + DK, minq:512],
                            start=True, stop=True,
                            skip_group_check=True)
                    pt_ = ptpool.tile([128, 2, 512], BF16, name="pt", tag="pt")
                    nc.scalar.activation(pt_[:, :, minq:512],
                                         pss[:, :, minq:512], Exp)
                    if di >= 0:
                        # Causal mask: zero the upper triangle of the exp'd
                        # diagonal block (both heads in one DVE multiply).
                        nc.vector.tensor_mul(
                            pt_[:, :, minq:minq + 128],
                            pt_[:, :, minq:minq + 128], trib_sb[:])
                    cur["pts"].append((kb, pt_, minq))
                    if last and not bstate["pending"]:
                        emit_avs(cur, kb)  # self-AVs trail exp by one block
                bstate["pending"].append(cur)
                maxlag = 0 if last else lag
                while len(bstate["pending"]) > maxlag:
                    head = bstate["pending"].pop(0)
                    emit_avs(head, 1 << 30)
                if last:
                    flush_norm2()

            def phase_c(qb):
                for mi in range(4):
                    m = 4 * qb + mi
                    msl = slice(m * 128, (m + 1) * 128)
                    for n2 in range(2):
                        nsl = slice(n2 * 512, (n2 + 1) * 512)
                        ps = mpool.tile([128, 512], F32, name="mp", tag="mp")
                        for db in range(NDB):
                            nc.tensor.matmul(
                                ps[:], OTc[db][qb][:, mi * 128:(mi + 1) * 128],
                                wo_sb[:, db, nsl],
                                start=(db == 0), stop=(db == NDB - 1))
                        ot = otpool.tile([128, 512], F32, name="ob", tag="ob")
                        nc.vector.tensor_add(ot[:], ps[:], bob_sb[:, nsl])
                        nc.sync.dma_start(out_d[msl, nsl], ot[:])

            xts0 = phase_a_dma(0, first=True)
            for t, dt_ in [(trib_sb, trib), (onesr_sb, onesr),
                           (bob_sb, bob), (wo_sb, wo)]:
                nc.sync.dma_start(t[:], dt_[:])
            phase_a_compute(0, xts0)
            # qb=0: lag 2 + slow AV pacing -- the V DMAs are still landing,
            # so AV matmuls must not sit in the PE FIFO ahead of ready
            # score matmuls.
            b_iter(0, 0, lag=2, pace_delay=2)
            xts1 = phase_a_dma(1)
            for hp in range(1, NDB):
                b_iter(0, hp, lag=2, pace_delay=2)
            phase_a_compute(1, xts1)
            b_iter(1, 0)          # drains+norms (0,2),(0,3) -> OTc[*][0] done
            xts2 = phase_a_dma(2)
            phase_c(0)
            for hp in range(1, NDB):
                b_iter(1, hp)
            phase_a_compute(2, xts2)
            b_iter(2, 0)
            xts3 = phase_a_dma(3)
            phase_c(1)
            for hp in range(1, NDB):
                b_iter(2, hp)
            phase_a_compute(3, xts3)
            b_iter(3, 0)
            phase_c(2)
            b_iter(3, 1)
            b_iter(3, 2)
            b_iter(3, 3, last=True)
            phase_c(3)

    nc.finalize()
    return nc


def _to_pkt(a2d, nt):
    """[nt*128, N] -> [128, nt, N] (partition-major tiling of the first dim)."""
    n = a2d.shape[1]
    return np.ascontiguousarray(
        a2d.reshape(nt, 128, n).transpose(1, 0, 2))


def _make_in_maps(query, value, key, Wq, bq, Wk, bk, Wv, bv, Wo, bo):
    import ml_dtypes

    f32 = np.float32
    bf16 = ml_dtypes.bfloat16
    query = np.asarray(query, f32)
    value = np.asarray(value, f32)
    key = np.asarray(key, f32)
    Wq = np.asarray(Wq, f32); bq = np.asarray(bq, f32)
    Wk = np.asarray(Wk, f32); bk = np.asarray(bk, f32)
    Wv = np.asarray(Wv, f32); bv = np.asarray(bv, f32)
    Wo = np.asarray(Wo, f32); bo = np.asarray(bo, f32)

    p = np.arange(128)[:, None]
    j = np.arange(128)[None, :]
    tri01 = np.where(p > j, 0.0, 1.0).astype(bf16)          # [128, 128]
    trib = np.ascontiguousarray(
        np.broadcast_to(tri01[:, None, :], (128, 2, 128)))  # both heads

    xT = {}
    for nm, x in (("q", query), ("k", key), ("v", value)):
        xT[nm] = [_to_pkt(x[b].T.astype(bf16), NBK) for b in range(4)]

    in_maps = []
    for b in range(4):
        for g in range(2):
            sl = slice(g * DL, (g + 1) * DL)
            bo_loc = bo if g == 0 else np.zeros_like(bo)
            m = {
                "xq": xT["q"][b],
                "xk": xT["k"][b],
                "xv": xT["v"][b],
                "wq": _to_pkt((Wq[:, sl] / 8.0).astype(bf16), NBK),
                "wk": _to_pkt(Wk[:, sl].astype(bf16), NBK),
                "wv": _to_pkt(Wv[:, sl].astype(bf16), NBK),
                "wo": _to_pkt(Wo[sl, :].astype(bf16), NDB),
                "bqs": np.ascontiguousarray((bq[sl] / 8.0).reshape(NDB, 128).T),
                "bks": np.ascontiguousarray(bk[sl].reshape(NDB, 128).T),
                "bvb": np.ascontiguousarray(
                    np.broadcast_to(bv[sl][None, :], (128, DL))).astype(bf16),
                "bob": np.ascontiguousarray(
                    np.broadcast_to(bo_loc[None, :], (128, D))).astype(bf16),
                "trib": trib,
                "onesr": np.ones((1, DK), f32),
            }
            in_maps.append(m)
    return in_maps


def kernel_with_info(inputs, trace=False):
    from concourse.bass_utils import run_bass_kernel_spmd

    if "nc" not in _NC:
        _NC["nc"] = _build_nc()

    in_maps = _make_in_maps(**inputs)
    res = run_bass_kernel_spmd(_NC["nc"], in_maps, core_ids=list(range(8)),
                               trace=trace)
    out = np.empty((4, S, D), np.float32)
    for b in range(4):
        out[b] = res.results[2 * b]["out"] + res.results[2 * b + 1]["out"]
    return out, res


def kernel(**inputs):
    out, _ = kernel_with_info(inputs)
    return out


# revision 23
# speedup vs baseline: 1.0605x; 1.0605x over previous
"""Multi-head attention (B=4, S=2048, D=1024, H=16, causal) on 8 trn2 cores.

Sharding: core = b*2 + g  (b = batch 0..3, g = head-group 0..1, 8 heads each).
All matmul operands are bf16 (enables Fast Weight Load on LDWEIGHTS and halves
input DMA); PSUM accumulation stays fp32.

The kernel is emitted as an interleaved software pipeline over 512-wide
sequence chunks so the Tile scheduler can overlap everything:

    A(0) B(0) A(1) C(0) B(1) A(2) C(1) B(2) A(3) C(2) B(3) C(3)

  A(s): Q^T/K^T projections for chunk s (d-major, per-chunk tiles
        QTc/KTc[db][s] = [128 d, 512 s]) and V projection for the four
        128-row key blocks of chunk s (s-major VT[kb] = [128 s, 8 h, 64+1]
        with an appended ones column per head -- the AV matmul then emits
        softmax denominators in PSUM row 64 for free).
  B(qb): for each head pair hp: transposed scores S^T = K_h Q_h^T per 128-key
        block, both heads row-tiled into disjoint PE row groups (concurrent)
        into one [128,2,512] PSUM tile; one Exp ACTIVATE per block covers
        both heads; causal masking = DVE multiply of the diagonal block's
        exp'd probabilities by a 0/1 lower-triangle tile (no PE mask matmul);
        O^T accumulated over key blocks into a [128,2,512] PSUM pair drawn
        from the same 3-slot pool (so the next pair's AV can start while the
        previous normalizes); normalization = DVE reciprocal of the PSUM
        denominator row + GpSimd partition-broadcast + DVE multiply into
        OTc[hp][qb] (bf16).  No PE instructions in the normalization path.
  C(qb): out[qb chunk] = OTc^T @ Wo, bias added by DVE during the PSUM->SBUF
        copy (pre-broadcast [128,D] bias tile), DMA to HBM.

Other details: ~60 warmup matmuls on a scratch tile at kernel start keep the
PE HAM clock-gate warm while the initial DMAs land; the first chunk's
weight/activation DMAs are split per contraction tile across the 16 DMA
queues so the first projection starts as early as possible.

Host: input transpose/bf16-cast + shard; the g-pair partial sum (row-parallel
Wo all-reduce) happens at gather time.
"""

import numpy as np

S = 2048
D = 1024
DL = 512          # local head dims per core (8 heads x 64)
HL = 8            # local heads
DK = 64
NBK = D // 128    # contraction tiles for projections
NDB = DL // 128   # d-out blocks (head pairs)
NQ = S // 512     # q blocks
NBS = S // 128    # s tiles / key blocks

_NC = {}


def _build_nc():
    import concourse.bass as bass
    import concourse.mybir as mybir
    import concourse.tile as tile
    from concourse import bacc

    F32 = mybir.dt.float32
    F32R = mybir.dt.float32r
    BF16 = mybir.dt.bfloat16
    Exp = mybir.ActivationFunctionType.Exp

    nc = bacc.Bacc(None)

    xq = nc.dram_tensor("xq", [128, NBK, S], BF16, kind="ExternalInput")
    xk = nc.dram_tensor("xk", [128, NBK, S], BF16, kind="ExternalInput")
    xv = nc.dram_tensor("xv", [128, NBK, S], BF16, kind="ExternalInput")
    wq = nc.dram_tensor("wq", [128, NBK, DL], BF16, kind="ExternalInput")
    wk = nc.dram_tensor("wk", [128, NBK, DL], BF16, kind="ExternalInput")
    wv = nc.dram_tensor("wv", [128, NBK, DL], BF16, kind="ExternalInput")
    wo = nc.dram_tensor("wo", [128, NDB, D], BF16, kind="ExternalInput")
    bqs = nc.dram_tensor("bqs", [128, NDB], F32, kind="ExternalInput")
    bks = nc.dram_tensor("bks", [128, NDB], F32, kind="ExternalInput")
    bvb = nc.dram_tensor("bvb", [128, DL], BF16, kind="ExternalInput")
    bob = nc.dram_tensor("bob", [128, D], BF16, kind="ExternalInput")
    trib = nc.dram_tensor("trib", [128, 2, 128], BF16, kind="ExternalInput")
    onesr = nc.dram_tensor("onesr", [1, DK], F32R, kind="ExternalInput")
    out_d = nc.dram_tensor("out", [S, D], F32, kind="ExternalOutput")

    with tile.TileContext(nc) as tc, nc.allow_low_precision(
            reason="bf16 matmul operands are intended"):
        with (
            tc.tile_pool(name="const", bufs=1) as cpool,
            tc.tile_pool(name="res", bufs=1) as rpool,
            tc.tile_pool(name="xt", bufs=3) as xpool,
            tc.tile_pool(name="pt", bufs=18) as ptpool,
            tc.tile_pool(name="rc", bufs=2) as rcpool,
            tc.tile_pool(name="rb", bufs=2) as rbpool,
            tc.tile_pool(name="ot", bufs=3) as otpool,
            tc.tile_pool(name="mp", bufs=2, space="PSUM") as mpool,
            tc.tile_pool(name="bp", bufs=3, space="PSUM") as bpool,
        ):
            # PE warmup: the HAM clock gate needs ~3.4us of sustained matmul
            # activity to lift the PE clock from 1.2 to 2.4 GHz.  Spend the
            # initial DMA wait running matmuls on a scratch tile so the real
            # projections start warm.
            warm_sb = cpool.tile([128, 128], BF16, name="warm", tag="warm")
            nc.vector.memset(warm_sb[:], 0.0)
            warm_ps = mpool.tile([128, 512], F32, name="mp", tag="mp")
            for _ in range(40):
                nc.tensor.matmul(warm_ps[:, 0:128], warm_sb[:], warm_sb[:],
                                 start=True, stop=True)

            # Startup order matters: the first projection only needs
            # bqs/bks + wq + the first xq chunk, so issue those DMAs first
            # (split per contraction tile so they spread across the 16 DMA
            # queues) and defer the B/C-phase constants until after the
            # first A chunk is emitted.
            bqs_sb = cpool.tile([128, NDB], F32, name="bqs", tag="bqs")
            bks_sb = cpool.tile([128, NDB], F32, name="bks", tag="bks")
            bvb_sb = cpool.tile([128, DL], BF16, name="bvb", tag="bvb")
            bob_sb = cpool.tile([128, D], BF16, name="bob", tag="bob")
            trib_sb = cpool.tile([128, 2, 128], BF16, name="trib", tag="trib")
            onesr_sb = cpool.tile([1, DK], F32R, name="onesr", tag="onesr")
            nc.sync.dma_start(bqs_sb[:], bqs[:])
            nc.sync.dma_start(bks_sb[:], bks[:])

            wq_sb = rpool.tile([128, NBK, DL], BF16, name="wq", tag="wq")
            wk_sb = rpool.tile([128, NBK, DL], BF16, name="wk", tag="wk")
            wv_sb = rpool.tile([128, NBK, DL], BF16, name="wv", tag="wv")
            wo_sb = rpool.tile([128, NDB, D], BF16, name="wo", tag="wo")

            QTc = [[rpool.tile([128, 512], BF16, name=f"QT{i}_{s}", tag=f"QT{i}_{s}")
                    for s in range(NQ)] for i in range(NDB)]
            KTc = [[rpool.tile([128, 512], BF16, name=f"KT{i}_{s}", tag=f"KT{i}_{s}")
                    for s in range(NQ)] for i in range(NDB)]
            # VT is flat [128, 520] = 8 heads x (64 V dims + a ones column);
            # the AV stationary operand is head h's 65-column window, whose
            # ones column makes PSUM row 64 the softmax denominator for free.
            # (65-col LDWEIGHTS gets no Fast Weight Load, but at 65 columns
            # it costs the same as a 128-col FWL load.)
            VT = [rpool.tile([128, HL * (DK + 1)], BF16,
                             name=f"VT{i}", tag=f"VT{i}")
                  for i in range(NBS)]
            OTc = [[rpool.tile([128, 512], BF16, name=f"OT{i}_{s}", tag=f"OT{i}_{s}")
                    for s in range(NQ)] for i in range(NDB)]

            def phase_a_dma(s, first=False):
                sl = slice(s * 512, (s + 1) * 512)
                xts = []
                for xd, w_sb_, wd in ((xq, wq_sb, wq), (xk, wk_sb, wk),
                                      (xv, wv_sb, wv)):
                    xt = xpool.tile([128, NBK, 512], BF16, name="xt", tag="xt")
                    if first:
                        # Interleave per-k weight and activation pieces so the
                        # k-loop of the first projection can start after the
                        # first pair lands and chase the rest.
                        for k in range(NBK):
                            nc.sync.dma_start(w_sb_[:, k, :], wd[:, k, :])
                            nc.sync.dma_start(xt[:, k, :], xd[:, k, sl])
                    else:
                        nc.sync.dma_start(xt[:], xd[:, :, sl])
                    xts.append(xt)
                if first:
                    nc.sync.dma_start(bvb_sb[:], bvb[:])
                return xts

            def phase_a_compute(s, xts, skip_v=False):
                # Head-pair 0 of Q/K and all of V before head-pairs 1..3, so
                # B(qb=s, hp=0) unblocks early.
                def proj_db(xt, w_sb_, b_sb, dst, db):
                    ps = mpool.tile([128, 512], F32, name="mp", tag="mp")
                    for k in range(NBK):
                        nc.tensor.matmul(
                            ps[:], w_sb_[:, k, db * 128:(db + 1) * 128],
                            xt[:, k, :], start=(k == 0), stop=(k == NBK - 1))
                    nc.vector.tensor_scalar_add(
                        dst[db][s][:], ps[:], b_sb[:, db:db + 1])

                proj_db(xts[0], wq_sb, bqs_sb, QTc, 0)
                proj_db(xts[1], wk_sb, bks_sb, KTc, 0)
                if not skip_v:
                    phase_a_v(s, xts)
                for db in range(1, NDB):
                    proj_db(xts[0], wq_sb, bqs_sb, QTc, db)
                    proj_db(xts[1], wk_sb, bks_sb, KTc, db)

            def phase_a_v(s, xts):
                for mi in range(4):
                    m = 4 * s + mi
                    ps = mpool.tile([128, 512], F32, name="mp", tag="mp")
                    for k in range(NBK):
                        nc.tensor.matmul(
                            ps[:], xts[2][:, k, mi * 128:(mi + 1) * 128],
                            wv_sb[:, k, :], start=(k == 0), stop=(k == NBK - 1))
                    vt3 = VT[m][:, 0:HL * (DK + 1)].rearrange(
                        "p (h c) -> p h c", c=DK + 1)
                    nc.vector.memset(vt3[:, :, DK:DK + 1], 1.0)
                    nc.vector.tensor_add(
                        vt3[:, :, 0:DK],
                        ps[:].rearrange("p (h c) -> p h c", c=DK),
                        bvb_sb[:].rearrange("p (h c) -> p h c", c=DK))

            # B runs as a software pipeline over (qb, hp) iterations: while
            # iteration i computes scores+exp (ACT-bound), the AV chain of
            # iteration i-1 -- whose pt tiles are all ready -- streams
            # back-to-back on the PE with no dependency waits.
            bstate = {"pending": [], "norm2": None, "norm2_age": 0}

            # The norm is split in two: stage 1 (DVE reciprocal of the PSUM
            # denominator row + GpSimd broadcast across the 64 d partitions)
            # fires as soon as an AV chain completes; stage 2 (the DVE
            # multiplies that read pso and free its PSUM slot) is deferred a
            # couple of key blocks so the GpSimd broadcast latency is hidden.
            def emit_norm1(it):
                # reciprocal_approx_fast is a bitwise custom DVE op -- it
                # only reads correctly from SBUF, so stage the PSUM
                # denominator row through an SBUF copy first.
                den = rcpool.tile([1, 2, 512], F32, name="rc", tag="rc")
                rden = rcpool.tile([1, 2, 512], F32, name="rc2", tag="rc2")
                rb = rbpool.tile([DK, 2, 512], F32, name="rb", tag="rb")
                # Per-h2 split so the first multiply (which frees the pso
                # PSUM slot) starts one broadcast earlier.
                for h2 in range(2):
                    nc.vector.tensor_copy(
                        den[:, h2, :], it["pso"][DK:DK + 1, h2, :])
                    nc.vector.reciprocal_approx_fast(
                        out=rden[:, h2, :], in_=den[:, h2, :])
                    nc.gpsimd.partition_broadcast(
                        rb[:, h2, :], rden[0:1, h2, :], channels=DK)
                it["rb"] = rb

            def emit_norm1_fast(it):
                # Tail fast path (last iteration): the GpSimd broadcast's
                # ~1.8us latency would sit serially in front of phase_c(3),
                # so broadcast on the (by then idle) PE instead, like the
                # denominator row itself: stationary ones [1, DK], moving
                # f32r denominator row.
                den = rcpool.tile([1, 2, 512], F32R, name="rcf", tag="rcf")
                nc.vector.tensor_copy(den[:], it["pso"][DK:DK + 1, :, :])
                rb = rbpool.tile([DK, 2, 512], F32, name="rbf", tag="rbf")
                for h2 in range(2):
                    psb = mpool.tile([DK, 512], F32, name="mp", tag="mp")
                    nc.tensor.matmul(psb[:], onesr_sb[0:1, :],
                                     den[0:1, h2, :], start=True, stop=True,
                                     skip_group_check=True)
                    nc.vector.reciprocal_approx_fast(
                        out=rb[:, h2, :], in_=psb[:])
                it["rb"] = rb

            def emit_norm2(it, split=False):
                hp, qb, pso, rb = it["hp"], it["qb"], it["pso"], it["rb"]
                if split:
                    # Column-split so phase_c's mi blocks unblock ASAP.
                    for q4 in range(4):
                        ql = slice(q4 * 128, (q4 + 1) * 128)
                        for h2 in range(2):
                            nc.vector.tensor_mul(
                                OTc[hp][qb][h2 * DK:(h2 + 1) * DK, ql],
                                pso[0:DK, h2, ql], rb[:, h2, ql])
                    return
                for h2 in range(2):
                    nc.vector.tensor_mul(
                        OTc[hp][qb][h2 * DK:(h2 + 1) * DK, :],
                        pso[0:DK, h2, :], rb[:, h2, :])

            def flush_norm2():
                it = bstate.get("norm2")
                if it is not None:
                    emit_norm2(it)
                    bstate["norm2"] = None

            def emit_avs(it, n):
                """Emit AV matmuls for iteration `it` up to index n; when the
                chain completes, normalize immediately so the pso slot frees
                as early as possible."""
                if it is None or it.get("done"):
                    return
                while it["emitted"] < min(n, it["kbmax"]):
                    kb, pt_, minq = it["pts"][it["emitted"]]
                    if it["pso"] is None:
                        it["pso"] = bpool.tile([128, 2, 512], F32,
                                               name="bp", tag="bp")
                    for h2 in range(2):
                        # Heads 0..6 use a 128-col stationary window (reaches
                        # into the next head's columns; PSUM rows 65.. are
                        # never read) to get Fast Weight Load; head 7's
                        # window would run off the tile so it uses 65 cols.
                        h = it["hp"] * 2 + h2
                        wc = 128 if h < HL - 1 else DK + 1
                        nc.tensor.matmul(
                            it["pso"][0:wc, h2, minq:512],
                            VT[kb][:, h * (DK + 1):h * (DK + 1) + wc],
                            pt_[:, h2, minq:512],
                            start=(kb == 0), stop=(kb == it["kbmax"] - 1),
                            skip_group_check=True)
                    it["emitted"] += 1
                if it["emitted"] == it["kbmax"]:
                    it["done"] = True
                    if it.get("fast"):
                        emit_norm1_fast(it)
                        emit_norm2(it, split=True)
                    else:
                        emit_norm1(it)
                        flush_norm2()
                        bstate["norm2"] = it
                        bstate["norm2_age"] = 0

            def b_iter(qb, hp, lag=1, pace_delay=0, last=False):
                kbmax = 4 * (qb + 1)
                cur = {"hp": hp, "qb": qb, "kbmax": kbmax, "fast": last,
                       "pts": [], "pso": None, "emitted": 0}
                for kb in range(kbmax):
                    # Ready AV matmuls go in front of the score matmul: the
                    # score may wait on a PSUM slot (exp pacing) and the PE
                    # queue is strict FIFO.
                    if bstate["pending"]:
                        head = bstate["pending"][0]
                        emit_avs(head, kb + 2 - pace_delay)
                        if head.get("done"):
                            bstate["pending"].pop(0)
                    if bstate.get("norm2") is not None:
                        bstate["norm2_age"] += 1
                        if bstate["norm2_age"] >= 2:
                            flush_norm2()
                    di = kb - 4 * qb
                    minq = 128 * di if di > 0 else 0
                    pss = bpool.tile([128, 2, 512], F32, name="bp", tag="bp")
                    for h2 in range(2):
                        base = h2 * DK
                        nc.tensor.matmul(
                            pss[:, h2, minq:512],
                            KTc[hp][kb // 4][base:base + DK,
                                             (kb % 4) * 128:(kb % 4 + 1) * 128],
                            QTc[hp][qb][base:base + DK, minq:512],
                            start=True, stop=True,
                            skip_group_check=True)
                    pt_ = ptpool.tile([128, 2, 512], BF16, name="pt", tag="pt")
                    nc.scalar.activation(pt_[:, :, minq:512],
                                         pss[:, :, minq:512], Exp)
                    if di >= 0:
                        # Causal mask: zero the upper triangle of the exp'd
                        # diagonal block (both heads in one DVE multiply).
                        nc.vector.tensor_mul(
                            pt_[:, :, minq:minq + 128],
                            pt_[:, :, minq:minq + 128], trib_sb[:])
                    cur["pts"].append((kb, pt_, minq))
                    if last and not bstate["pending"]:
                        emit_avs(cur, kb)  # self-AVs trail exp by one block
                bstate["pending"].append(cur)
                maxlag = 0 if last else lag
                while len(bstate["pending"]) > maxlag:
                    head = bstate["pending"].pop(0)
                    emit_avs(head, 1 << 30)
                if last:
                    flush_norm2()

            def phase_c(qb):
                for mi in range(4):
                    m = 4 * qb + mi
                    msl = slice(m * 128, (m + 1) * 128)
                    for n2 in range(2):
                        nsl = slice(n2 * 512, (n2 + 1) * 512)
                        ps = mpool.tile([128, 512], F32, name="mp", tag="mp")
                        for db in range(NDB):
                            nc.tensor.matmul(
                                ps[:], OTc[db][qb][:, mi * 128:(mi + 1) * 128],
                                wo_sb[:, db, nsl],
                                start=(db == 0), stop=(db == NDB - 1))
                        ot = otpool.tile([128, 512], F32, name="ob", tag="ob")
                        nc.vector.tensor_add(ot[:], ps[:], bob_sb[:, nsl])
                        nc.sync.dma_start(out_d[msl, nsl], ot[:])

            xts0 = phase_a_dma(0, first=True)
            for t, dt_ in [(trib_sb, trib), (onesr_sb, onesr),
                           (bob_sb, bob), (wo_sb, wo)]:
                nc.sync.dma_start(t[:], dt_[:])
            # Chunk 0: V projections wait on the last-priority xv/wv DMAs,
            # so emit them after the first score block -- otherwise they
            # block ready Q/K projections and scores in the PE FIFO.
            phase_a_compute(0, xts0, skip_v=True)
            # qb=0: lag 2 + slow AV pacing -- the V DMAs are still landing,
            # so AV matmuls must not sit in the PE FIFO ahead of ready
            # score matmuls.
            b_iter(0, 0, lag=2, pace_delay=2)
            phase_a_v(0, xts0)
            xts1 = phase_a_dma(1)
            for hp in range(1, NDB):
                b_iter(0, hp, lag=2, pace_delay=2)
            phase_a_compute(1, xts1)
            b_iter(1, 0)          # drains+norms (0,2),(0,3) -> OTc[*][0] done
            xts2 = phase_a_dma(2)
            phase_c(0)
            for hp in range(1, NDB):
                b_iter(1, hp)
            phase_a_compute(2, xts2)
            b_iter(2, 0)
            xts3 = phase_a_dma(3)
            phase_c(1)
            for hp in range(1, NDB):
                b_iter(2, hp)
            phase_a_compute(3, xts3)
            b_iter(3, 0)
            phase_c(2)
            b_iter(3, 1)
            b_iter(3, 2)
            b_iter(3, 3, last=True)
            phase_c(3)

    nc.finalize()
    return nc


def _to_pkt(a2d, nt):
    """[nt*128, N] -> [128, nt, N] (partition-major tiling of the first dim)."""
    n = a2d.shape[1]
    return np.ascontiguousarray(
        a2d.reshape(nt, 128, n).transpose(1, 0, 2))


def _make_in_maps(query, value, key, Wq, bq, Wk, bk, Wv, bv, Wo, bo):
    import ml_dtypes

    f32 = np.float32
    bf16 = ml_dtypes.bfloat16
    query = np.asarray(query, f32)
    value = np.asarray(value, f32)
    key = np.asarray(key, f32)
    Wq = np.asarray(Wq, f32); bq = np.asarray(bq, f32)
    Wk = np.asarray(Wk, f32); bk = np.asarray(bk, f32)
    Wv = np.asarray(Wv, f32); bv = np.asarray(bv, f32)
    Wo = np.asarray(Wo, f32); bo = np.asarray(bo, f32)

    p = np.arange(128)[:, None]
    j = np.arange(128)[None, :]
    tri01 = np.where(p > j, 0.0, 1.0).astype(bf16)          # [128, 128]
    trib = np.ascontiguousarray(
        np.broadcast_to(tri01[:, None, :], (128, 2, 128)))  # both heads

    xT = {}
    for nm, x in (("q", query), ("k", key), ("v", value)):
        xT[nm] = [_to_pkt(x[b].T.astype(bf16), NBK) for b in range(4)]

    in_maps = []
    for b in range(4):
        for g in range(2):
            sl = slice(g * DL, (g + 1) * DL)
            bo_loc = bo if g == 0 else np.zeros_like(bo)
            m = {
                "xq": xT["q"][b],
                "xk": xT["k"][b],
                "xv": xT["v"][b],
                "wq": _to_pkt((Wq[:, sl] / 8.0).astype(bf16), NBK),
                "wk": _to_pkt(Wk[:, sl].astype(bf16), NBK),
                "wv": _to_pkt(Wv[:, sl].astype(bf16), NBK),
                "wo": _to_pkt(Wo[sl, :].astype(bf16), NDB),
                "bqs": np.ascontiguousarray((bq[sl] / 8.0).reshape(NDB, 128).T),
                "bks": np.ascontiguousarray(bk[sl].reshape(NDB, 128).T),
                "bvb": np.ascontiguousarray(
                    np.broadcast_to(bv[sl][None, :], (128, DL))).astype(bf16),
                "bob": np.ascontiguousarray(
                    np.broadcast_to(bo_loc[None, :], (128, D))).astype(bf16),
                "trib": trib,
                "onesr": np.ones((1, DK), f32),
            }
            in_maps.append(m)
    return in_maps


def kernel_with_info(inputs, trace=False):
    from concourse.bass_utils import run_bass_kernel_spmd

    if "nc" not in _NC:
        _NC["nc"] = _build_nc()

    in_maps = _make_in_maps(**inputs)
    res = run_bass_kernel_spmd(_NC["nc"], in_maps, core_ids=list(range(8)),
                               trace=trace)
    out = np.empty((4, S, D), np.float32)
    for b in range(4):
        out[b] = res.results[2 * b]["out"] + res.results[2 * b + 1]["out"]
    return out, res


def kernel(**inputs):
    out, _ = kernel_with_info(inputs)
    return out


# revision 27
# speedup vs baseline: 1.0814x; 1.0197x over previous
"""Multi-head attention (B=4, S=2048, D=1024, H=16, causal) on 8 trn2 cores.

Sharding: core = b*2 + g  (b = batch 0..3, g = head-group 0..1, 8 heads each).
All matmul operands are bf16 (enables Fast Weight Load on LDWEIGHTS and halves
input DMA); PSUM accumulation stays fp32.

The kernel is emitted as an interleaved software pipeline over 512-wide
sequence chunks so the Tile scheduler can overlap everything:

    A(0) B(0) A(1) C(0) B(1) A(2) C(1) B(2) A(3) C(2) B(3) C(3)

  A(s): Q^T/K^T projections for chunk s (d-major, per-chunk tiles
        QTc/KTc[db][s] = [128 d, 512 s]) and V projection for the four
        128-row key blocks of chunk s (s-major VT[kb] = [128 s, 8 h, 64+1]
        with an appended ones column per head -- the AV matmul then emits
        softmax denominators in PSUM row 64 for free).
  B(qb): for each head pair hp: transposed scores S^T = K_h Q_h^T per 128-key
        block, both heads row-tiled into disjoint PE row groups (concurrent)
        into one [128,2,512] PSUM tile; one Exp ACTIVATE per block covers
        both heads; causal masking = DVE multiply of the diagonal block's
        exp'd probabilities by a 0/1 lower-triangle tile (no PE mask matmul);
        O^T accumulated over key blocks into a [128,2,512] PSUM pair drawn
        from the same 3-slot pool (so the next pair's AV can start while the
        previous normalizes); normalization = DVE reciprocal of the PSUM
        denominator row + GpSimd partition-broadcast + DVE multiply into
        OTc[hp][qb] (bf16).  No PE instructions in the normalization path.
  C(qb): out[qb chunk] = OTc^T @ Wo, bias added by DVE during the PSUM->SBUF
        copy (pre-broadcast [128,D] bias tile), DMA to HBM.

Other details: ~60 warmup matmuls on a scratch tile at kernel start keep the
PE HAM clock-gate warm while the initial DMAs land; the first chunk's
weight/activation DMAs are split per contraction tile across the 16 DMA
queues so the first projection starts as early as possible.

Host: input transpose/bf16-cast + shard; the g-pair partial sum (row-parallel
Wo all-reduce) happens at gather time.
"""

import numpy as np

S = 2048
D = 1024
DL = 512          # local head dims per core (8 heads x 64)
HL = 8            # local heads
DK = 64
NBK = D // 128    # contraction tiles for projections
NDB = DL // 128   # d-out blocks (head pairs)
NQ = S // 512     # q blocks
NBS = S // 128    # s tiles / key blocks

_NC = {}


def _build_nc():
    import concourse.bass as bass
    import concourse.mybir as mybir
    import concourse.tile as tile
    from concourse import bacc

    F32 = mybir.dt.float32
    F32R = mybir.dt.float32r
    BF16 = mybir.dt.bfloat16
    Exp = mybir.ActivationFunctionType.Exp

    nc = bacc.Bacc(None)

    xq = nc.dram_tensor("xq", [128, NBK, S], BF16, kind="ExternalInput")
    xk = nc.dram_tensor("xk", [128, NBK, S], BF16, kind="ExternalInput")
    xv = nc.dram_tensor("xv", [128, NBK, S], BF16, kind="ExternalInput")
    wq = nc.dram_tensor("wq", [128, NBK, DL], BF16, kind="ExternalInput")
    wk = nc.dram_tensor("wk", [128, NBK, DL], BF16, kind="ExternalInput")
    wv = nc.dram_tensor("wv", [128, NBK, DL], BF16, kind="ExternalInput")
    wo = nc.dram_tensor("wo", [128, NDB, D], BF16, kind="ExternalInput")
    bqs = nc.dram_tensor("bqs", [128, NDB], F32, kind="ExternalInput")
    bks = nc.dram_tensor("bks", [128, NDB], F32, kind="ExternalInput")
    bvb = nc.dram_tensor("bvb", [128, DL], BF16, kind="ExternalInput")
    bob = nc.dram_tensor("bob", [128, D], BF16, kind="ExternalInput")
    trib = nc.dram_tensor("trib", [128, 2, 128], BF16, kind="ExternalInput")
    onesr = nc.dram_tensor("onesr", [1, DK], F32R, kind="ExternalInput")
    out_d = nc.dram_tensor("out", [S, D], F32, kind="ExternalOutput")

    with tile.TileContext(nc) as tc, nc.allow_low_precision(
            reason="bf16 matmul operands are intended"):
        with (
            tc.tile_pool(name="const", bufs=1) as cpool,
            tc.tile_pool(name="res", bufs=1) as rpool,
            tc.tile_pool(name="xt", bufs=3) as xpool,
            tc.tile_pool(name="pt", bufs=18) as ptpool,
            tc.tile_pool(name="rc", bufs=2) as rcpool,
            tc.tile_pool(name="rb", bufs=2) as rbpool,
            tc.tile_pool(name="ot", bufs=3) as otpool,
            tc.tile_pool(name="mp", bufs=2, space="PSUM") as mpool,
            tc.tile_pool(name="bp", bufs=3, space="PSUM") as bpool,
        ):
            # PE warmup: the HAM clock gate needs ~3.4us of sustained matmul
            # activity to lift the PE clock from 1.2 to 2.4 GHz.  Spend the
            # initial DMA wait running matmuls on a scratch tile so the real
            # projections start warm.
            warm_sb = cpool.tile([128, 128], BF16, name="warm", tag="warm")
            nc.vector.memset(warm_sb[:], 0.0)
            warm_ps = mpool.tile([128, 512], F32, name="mp", tag="mp")
            for _ in range(40):
                nc.tensor.matmul(warm_ps[:, 0:128], warm_sb[:], warm_sb[:],
                                 start=True, stop=True)

            # Startup order matters: the first projection only needs
            # bqs/bks + wq + the first xq chunk, so issue those DMAs first
            # (split per contraction tile so they spread across the 16 DMA
            # queues) and defer the B/C-phase constants until after the
            # first A chunk is emitted.
            bqs_sb = cpool.tile([128, NDB], F32, name="bqs", tag="bqs")
            bks_sb = cpool.tile([128, NDB], F32, name="bks", tag="bks")
            bvb_sb = cpool.tile([128, DL], BF16, name="bvb", tag="bvb")
            bob_sb = cpool.tile([128, D], BF16, name="bob", tag="bob")
            trib_sb = cpool.tile([128, 2, 128], BF16, name="trib", tag="trib")
            onesr_sb = cpool.tile([1, DK], F32R, name="onesr", tag="onesr")
            nc.sync.dma_start(bqs_sb[:], bqs[:])
            nc.sync.dma_start(bks_sb[:], bks[:])

            wq_sb = rpool.tile([128, NBK, DL], BF16, name="wq", tag="wq")
            wk_sb = rpool.tile([128, NBK, DL], BF16, name="wk", tag="wk")
            wv_sb = rpool.tile([128, NBK, DL], BF16, name="wv", tag="wv")
            wo_sb = rpool.tile([128, NDB, D], BF16, name="wo", tag="wo")

            QTc = [[rpool.tile([128, 512], BF16, name=f"QT{i}_{s}", tag=f"QT{i}_{s}")
                    for s in range(NQ)] for i in range(NDB)]
            KTc = [[rpool.tile([128, 512], BF16, name=f"KT{i}_{s}", tag=f"KT{i}_{s}")
                    for s in range(NQ)] for i in range(NDB)]
            # VT is flat [128, 520] = 8 heads x (64 V dims + a ones column);
            # the AV stationary operand is head h's 65-column window, whose
            # ones column makes PSUM row 64 the softmax denominator for free.
            # (65-col LDWEIGHTS gets no Fast Weight Load, but at 65 columns
            # it costs the same as a 128-col FWL load.)
            VT = [rpool.tile([128, HL * (DK + 1)], BF16,
                             name=f"VT{i}", tag=f"VT{i}")
                  for i in range(NBS)]
            OTc = [[rpool.tile([128, 512], BF16, name=f"OT{i}_{s}", tag=f"OT{i}_{s}")
                    for s in range(NQ)] for i in range(NDB)]

            def phase_a_dma(s, first=False):
                sl = slice(s * 512, (s + 1) * 512)
                xts = []
                for xd, w_sb_, wd in ((xq, wq_sb, wq), (xk, wk_sb, wk),
                                      (xv, wv_sb, wv)):
                    xt = xpool.tile([128, NBK, 512], BF16, name="xt", tag="xt")
                    if first:
                        # Interleave per-k weight and activation pieces so the
                        # k-loop of the first projection can start after the
                        # first pair lands and chase the rest.
                        for k in range(NBK):
                            nc.sync.dma_start(w_sb_[:, k, :], wd[:, k, :])
                            nc.sync.dma_start(xt[:, k, :], xd[:, k, sl])
                    else:
                        # Two pieces per tensor -> lands on more DMA queues.
                        nc.sync.dma_start(xt[:, 0:NBK // 2, :],
                                          xd[:, 0:NBK // 2, sl])
                        nc.sync.dma_start(xt[:, NBK // 2:, :],
                                          xd[:, NBK // 2:, sl])
                    xts.append(xt)
                if first:
                    nc.sync.dma_start(bvb_sb[:], bvb[:])
                return xts

            def proj_db(xts, s, which, db):
                xt = xts[which]
                w_sb_ = (wq_sb, wk_sb)[which]
                b_sb = (bqs_sb, bks_sb)[which]
                dst = (QTc, KTc)[which]
                ps = mpool.tile([128, 512], F32, name="mp", tag="mp")
                for k in range(NBK):
                    nc.tensor.matmul(
                        ps[:], w_sb_[:, k, db * 128:(db + 1) * 128],
                        xt[:, k, :], start=(k == 0), stop=(k == NBK - 1))
                nc.vector.tensor_scalar_add(
                    dst[db][s][:], ps[:], b_sb[:, db:db + 1])

            def proj_v(xts, s, mi):
                m = 4 * s + mi
                ps = mpool.tile([128, 512], F32, name="mp", tag="mp")
                for k in range(NBK):
                    nc.tensor.matmul(
                        ps[:], xts[2][:, k, mi * 128:(mi + 1) * 128],
                        wv_sb[:, k, :], start=(k == 0), stop=(k == NBK - 1))
                vt3 = VT[m][:, 0:HL * (DK + 1)].rearrange(
                    "p (h c) -> p h c", c=DK + 1)
                nc.vector.memset(vt3[:, :, DK:DK + 1], 1.0)
                nc.vector.tensor_add(
                    vt3[:, :, 0:DK],
                    ps[:].rearrange("p (h c) -> p h c", c=DK),
                    bvb_sb[:].rearrange("p (h c) -> p h c", c=DK))

            def phase_a_compute(s, xts, skip_v=False):
                # Head-pair 0 of Q/K and all of V before head-pairs 1..3, so
                # B(qb=s, hp=0) unblocks early.
                proj_db(xts, s, 0, 0)
                proj_db(xts, s, 1, 0)
                if not skip_v:
                    phase_a_v(s, xts)
                for db in range(1, NDB):
                    proj_db(xts, s, 0, db)
                    proj_db(xts, s, 1, db)

            def phase_a_v(s, xts):
                for mi in range(4):
                    proj_v(xts, s, mi)

            # Filler queue: next-chunk projection groups get sprinkled into
            # the B phase emission (one 8-matmul group every other key block
            # for the later head pairs), so the exp-paced stretches of the PE
            # FIFO always contain ready work ahead of a waiting score pair.
            filler = {"units": []}

            def filler_load(s, xts):
                u = filler["units"]
                u.append(lambda: proj_db(xts, s, 0, 0))
                u.append(lambda: proj_db(xts, s, 1, 0))
                for mi in range(4):
                    u.append(lambda mi=mi: proj_v(xts, s, mi))
                for db in range(1, NDB):
                    u.append(lambda db=db: proj_db(xts, s, 0, db))
                    u.append(lambda db=db: proj_db(xts, s, 1, db))

            def pump_filler(n=1):
                while n > 0 and filler["units"]:
                    filler["units"].pop(0)()
                    n -= 1

            def flush_filler():
                pump_filler(1 << 30)

            # B runs as a software pipeline over (qb, hp) iterations: while
            # iteration i computes scores+exp (ACT-bound), the AV chain of
            # iteration i-1 -- whose pt tiles are all ready -- streams
            # back-to-back on the PE with no dependency waits.
            bstate = {"pending": [], "norm2": None, "norm2_age": 0}

            # The norm is split in two: stage 1 (DVE reciprocal of the PSUM
            # denominator row + GpSimd broadcast across the 64 d partitions)
            # fires as soon as an AV chain completes; stage 2 (the DVE
            # multiplies that read pso and free its PSUM slot) is deferred a
            # couple of key blocks so the GpSimd broadcast latency is hidden.
            def emit_norm1(it):
                # reciprocal_approx_fast is a bitwise custom DVE op -- it
                # only reads correctly from SBUF, so stage the PSUM
                # denominator row through an SBUF copy first.
                den = rcpool.tile([1, 2, 512], F32, name="rc", tag="rc")
                rden = rcpool.tile([1, 2, 512], F32, name="rc2", tag="rc2")
                rb = rbpool.tile([DK, 2, 512], F32, name="rb", tag="rb")
                # Per-h2 split so the first multiply (which frees the pso
                # PSUM slot) starts one broadcast earlier.
                for h2 in range(2):
                    nc.vector.tensor_copy(
                        den[:, h2, :], it["pso"][DK:DK + 1, h2, :])
                    nc.vector.reciprocal_approx_fast(
                        out=rden[:, h2, :], in_=den[:, h2, :])
                    nc.gpsimd.partition_broadcast(
                        rb[:, h2, :], rden[0:1, h2, :], channels=DK)
                it["rb"] = rb

            def emit_norm1_fast(it):
                # Tail fast path (last iteration): the GpSimd broadcast's
                # ~1.8us latency would sit serially in front of phase_c(3),
                # so broadcast on the (by then idle) PE instead, like the
                # denominator row itself: stationary ones [1, DK], moving
                # f32r denominator row.
                den = rcpool.tile([1, 2, 512], F32R, name="rcf", tag="rcf")
                nc.vector.tensor_copy(den[:], it["pso"][DK:DK + 1, :, :])
                rb = rbpool.tile([DK, 2, 512], F32, name="rbf", tag="rbf")
                for h2 in range(2):
                    psb = mpool.tile([DK, 512], F32, name="mp", tag="mp")
                    nc.tensor.matmul(psb[:], onesr_sb[0:1, :],
                                     den[0:1, h2, :], start=True, stop=True,
                                     skip_group_check=True)
                    nc.vector.reciprocal_approx_fast(
                        out=rb[:, h2, :], in_=psb[:])
                it["rb"] = rb

            def emit_norm2(it, split=False):
                hp, qb, pso, rb = it["hp"], it["qb"], it["pso"], it["rb"]
                if split:
                    # Column-split so phase_c's mi blocks unblock ASAP.
                    for q4 in range(4):
                        ql = slice(q4 * 128, (q4 + 1) * 128)
                        for h2 in range(2):
                            nc.vector.tensor_mul(
                                OTc[hp][qb][h2 * DK:(h2 + 1) * DK, ql],
                                pso[0:DK, h2, ql], rb[:, h2, ql])
                    return
                for h2 in range(2):
                    nc.vector.tensor_mul(
                        OTc[hp][qb][h2 * DK:(h2 + 1) * DK, :],
                        pso[0:DK, h2, :], rb[:, h2, :])

            def flush_norm2():
                it = bstate.get("norm2")
                if it is not None:
                    emit_norm2(it)
                    bstate["norm2"] = None

            def emit_avs(it, n):
                """Emit AV matmuls for iteration `it` up to index n; when the
                chain completes, normalize immediately so the pso slot frees
                as early as possible."""
                if it is None or it.get("done"):
                    return
                while it["emitted"] < min(n, it["kbmax"]):
                    kb, pt_, minq = it["pts"][it["emitted"]]
                    if it["pso"] is None:
                        it["pso"] = bpool.tile([128, 2, 512], F32,
                                               name="bp", tag="bp")
                    for h2 in range(2):
                        # Heads 0..6 use a 128-col stationary window (reaches
                        # into the next head's columns; PSUM rows 65.. are
                        # never read) to get Fast Weight Load; head 7's
                        # window would run off the tile so it uses 65 cols.
                        h = it["hp"] * 2 + h2
                        wc = 128 if h < HL - 1 else DK + 1
                        nc.tensor.matmul(
                            it["pso"][0:wc, h2, minq:512],
                            VT[kb][:, h * (DK + 1):h * (DK + 1) + wc],
                            pt_[:, h2, minq:512],
                            start=(kb == 0), stop=(kb == it["kbmax"] - 1),
                            skip_group_check=True)
                    it["emitted"] += 1
                if it["emitted"] == it["kbmax"]:
                    it["done"] = True
                    if it.get("fast"):
                        emit_norm1_fast(it)
                        emit_norm2(it, split=True)
                    else:
                        emit_norm1(it)
                        flush_norm2()
                        bstate["norm2"] = it
                        bstate["norm2_age"] = 0

            def b_iter(qb, hp, lag=1, pace_delay=0, last=False, fill=False):
                kbmax = 4 * (qb + 1)
                cur = {"hp": hp, "qb": qb, "kbmax": kbmax,
                       "fast": last or hp == NDB - 1,
                       "pts": [], "pso": None, "emitted": 0}
                for kb in range(kbmax):
                    # Ready AV matmuls go in front of the score matmul: the
                    # score may wait on a PSUM slot (exp pacing) and the PE
                    # queue is strict FIFO.
                    if bstate["pending"]:
                        head = bstate["pending"][0]
                        emit_avs(head, kb + 2 - pace_delay)
                        if head.get("done"):
                            bstate["pending"].pop(0)
                    if fill and kb % 2 == 1:
                        pump_filler(1)
                    if bstate.get("norm2") is not None:
                        bstate["norm2_age"] += 1
                        if bstate["norm2_age"] >= 2:
                            flush_norm2()
                    di = kb - 4 * qb
                    minq = 128 * di if di > 0 else 0
                    pss = bpool.tile([128, 2, 512], F32, name="bp", tag="bp")
                    for h2 in range(2):
                        base = h2 * DK
                        nc.tensor.matmul(
                            pss[:, h2, minq:512],
                            KTc[hp][kb // 4][base:base + DK,
                                             (kb % 4) * 128:(kb % 4 + 1) * 128],
                            QTc[hp][qb][base:base + DK, minq:512],
                            start=True, stop=True,
                            skip_group_check=True)
                    pt_ = ptpool.tile([128, 2, 512], BF16, name="pt", tag="pt")
                    nc.scalar.activation(pt_[:, :, minq:512],
                                         pss[:, :, minq:512], Exp)
                    if di >= 0:
                        # Causal mask: zero the upper triangle of the exp'd
                        # diagonal block (both heads in one DVE multiply).
                        nc.vector.tensor_mul(
                            pt_[:, :, minq:minq + 128],
                            pt_[:, :, minq:minq + 128], trib_sb[:])
                    cur["pts"].append((kb, pt_, minq))
                    if last and not bstate["pending"]:
                        emit_avs(cur, kb)  # self-AVs trail exp by one block
                bstate["pending"].append(cur)
                maxlag = 0 if last else lag
                while len(bstate["pending"]) > maxlag:
                    head = bstate["pending"].pop(0)
                    emit_avs(head, 1 << 30)
                if last:
                    flush_norm2()

            def phase_c(qb):
                for mi in range(4):
                    m = 4 * qb + mi
                    msl = slice(m * 128, (m + 1) * 128)
                    for n2 in range(2):
                        nsl = slice(n2 * 512, (n2 + 1) * 512)
                        ps = mpool.tile([128, 512], F32, name="mp", tag="mp")
                        for db in range(NDB):
                            nc.tensor.matmul(
                                ps[:], OTc[db][qb][:, mi * 128:(mi + 1) * 128],
                                wo_sb[:, db, nsl],
                                start=(db == 0), stop=(db == NDB - 1))
                        ot = otpool.tile([128, 512], F32, name="ob", tag="ob")
                        nc.vector.tensor_add(ot[:], ps[:], bob_sb[:, nsl])
                        nc.sync.dma_start(out_d[msl, nsl], ot[:])

            xts0 = phase_a_dma(0, first=True)
            for t, dt_ in [(trib_sb, trib), (onesr_sb, onesr),
                           (bob_sb, bob), (wo_sb, wo)]:
                nc.sync.dma_start(t[:], dt_[:])
            # Chunk 0: V projections wait on the last-priority xv/wv DMAs,
            # so emit them after the first score block -- otherwise they
            # block ready Q/K projections and scores in the PE FIFO.
            phase_a_compute(0, xts0, skip_v=True)
            # qb=0: lag 2 + slow AV pacing -- the V DMAs are still landing,
            # so AV matmuls must not sit in the PE FIFO ahead of ready
            # score matmuls.
            b_iter(0, 0, lag=2, pace_delay=2)
            phase_a_v(0, xts0)
            xts1 = phase_a_dma(1)
            for hp in range(1, NDB):
                b_iter(0, hp, lag=2, pace_delay=2)
            phase_a_compute(1, xts1)
            b_iter(1, 0)          # drains+norms (0,2),(0,3) -> OTc[*][0] done
            xts2 = phase_a_dma(2)
            phase_c(0)
            b_iter(1, 1)
            filler_load(2, xts2)
            b_iter(1, 2, fill=True)
            b_iter(1, 3, fill=True)
            flush_filler()        # chunk-2 projections not yet sprinkled
            b_iter(2, 0)
            xts3 = phase_a_dma(3)
            phase_c(1)
            b_iter(2, 1)
            filler_load(3, xts3)
            b_iter(2, 2, fill=True)
            b_iter(2, 3, fill=True)
            flush_filler()
            b_iter(3, 0)
            phase_c(2)
            b_iter(3, 1)
            b_iter(3, 2)
            b_iter(3, 3, last=True)
            phase_c(3)

    nc.finalize()
    return nc


def _to_pkt(a2d, nt):
    """[nt*128, N] -> [128, nt, N] (partition-major tiling of the first dim)."""
    n = a2d.shape[1]
    return np.ascontiguousarray(
        a2d.reshape(nt, 128, n).transpose(1, 0, 2))


def _make_in_maps(query, value, key, Wq, bq, Wk, bk, Wv, bv, Wo, bo):
    import ml_dtypes

    f32 = np.float32
    bf16 = ml_dtypes.bfloat16
    query = np.asarray(query, f32)
    value = np.asarray(value, f32)
    key = np.asarray(key, f32)
    Wq = np.asarray(Wq, f32); bq = np.asarray(bq, f32)
    Wk = np.asarray(Wk, f32); bk = np.asarray(bk, f32)
    Wv = np.asarray(Wv, f32); bv = np.asarray(bv, f32)
    Wo = np.asarray(Wo, f32); bo = np.asarray(bo, f32)

    p = np.arange(128)[:, None]
    j = np.arange(128)[None, :]
    tri01 = np.where(p > j, 0.0, 1.0).astype(bf16)          # [128, 128]
    trib = np.ascontiguousarray(
        np.broadcast_to(tri01[:, None, :], (128, 2, 128)))  # both heads

    xT = {}
    for nm, x in (("q", query), ("k", key), ("v", value)):
        xT[nm] = [_to_pkt(x[b].T.astype(bf16), NBK) for b in range(4)]

    in_maps = []
    for b in range(4):
        for g in range(2):
            sl = slice(g * DL, (g + 1) * DL)
            bo_loc = bo if g == 0 else np.zeros_like(bo)
            m = {
                "xq": xT["q"][b],
                "xk": xT["k"][b],
                "xv": xT["v"][b],
                "wq": _to_pkt((Wq[:, sl] / 8.0).astype(bf16), NBK),
                "wk": _to_pkt(Wk[:, sl].astype(bf16), NBK),
                "wv": _to_pkt(Wv[:, sl].astype(bf16), NBK),
                "wo": _to_pkt(Wo[sl, :].astype(bf16), NDB),
                "bqs": np.ascontiguousarray((bq[sl] / 8.0).reshape(NDB, 128).T),
                "bks": np.ascontiguousarray(bk[sl].reshape(NDB, 128).T),
                "bvb": np.ascontiguousarray(
                    np.broadcast_to(bv[sl][None, :], (128, DL))).astype(bf16),
                "bob": np.ascontiguousarray(
                    np.broadcast_to(bo_loc[None, :], (128, D))).astype(bf16),
                "trib": trib,
                "onesr": np.ones((1, DK), f32),
            }
            in_maps.append(m)
    return in_maps


def kernel_with_info(inputs, trace=False):
    from concourse.bass_utils import run_bass_kernel_spmd

    if "nc" not in _NC:
        _NC["nc"] = _build_nc()

    in_maps = _make_in_maps(**inputs)
    res = run_bass_kernel_spmd(_NC["nc"], in_maps, core_ids=list(range(8)),
                               trace=trace)
    out = np.empty((4, S, D), np.float32)
    for b in range(4):
        out[b] = res.results[2 * b]["out"] + res.results[2 * b + 1]["out"]
    return out, res


def kernel(**inputs):
    out, _ = kernel_with_info(inputs)
    return out


# revision 28
# speedup vs baseline: 1.1085x; 1.0250x over previous
"""Multi-head attention (B=4, S=2048, D=1024, H=16, causal) on 8 trn2 cores.

Sharding: core = b*2 + g  (b = batch 0..3, g = head-group 0..1, 8 heads each).
All matmul operands are bf16 (enables Fast Weight Load on LDWEIGHTS and halves
input DMA); PSUM accumulation stays fp32.

The kernel is emitted as an interleaved software pipeline over 512-wide
sequence chunks so the Tile scheduler can overlap everything:

    A(0) B(0) A(1) C(0) B(1) A(2) C(1) B(2) A(3) C(2) B(3) C(3)

  A(s): Q^T/K^T projections for chunk s (d-major, per-chunk tiles
        QTc/KTc[db][s] = [128 d, 512 s]) and V projection for the four
        128-row key blocks of chunk s (s-major VT[kb] = [128 s, 8 h, 64+1]
        with an appended ones column per head -- the AV matmul then emits
        softmax denominators in PSUM row 64 for free).
  B(qb): for each head pair hp: transposed scores S^T = K_h Q_h^T per 128-key
        block, both heads row-tiled into disjoint PE row groups (concurrent)
        into one [128,2,512] PSUM tile; one Exp ACTIVATE per block covers
        both heads; causal masking = DVE multiply of the diagonal block's
        exp'd probabilities by a 0/1 lower-triangle tile (no PE mask matmul);
        O^T accumulated over key blocks into a [128,2,512] PSUM pair drawn
        from the same 3-slot pool (so the next pair's AV can start while the
        previous normalizes); normalization = DVE reciprocal of the PSUM
        denominator row + GpSimd partition-broadcast + DVE multiply into
        OTc[hp][qb] (bf16).  No PE instructions in the normalization path.
  C(qb): out[qb chunk] = OTc^T @ Wo, bias added by DVE during the PSUM->SBUF
        copy (pre-broadcast [128,D] bias tile), DMA to HBM.

Other details: ~60 warmup matmuls on a scratch tile at kernel start keep the
PE HAM clock-gate warm while the initial DMAs land; the first chunk's
weight/activation DMAs are split per contraction tile across the 16 DMA
queues so the first projection starts as early as possible.

Host: input transpose/bf16-cast + shard; the g-pair partial sum (row-parallel
Wo all-reduce) happens at gather time.
"""

import numpy as np

S = 2048
D = 1024
DL = 512          # local head dims per core (8 heads x 64)
HL = 8            # local heads
DK = 64
NBK = D // 128    # contraction tiles for projections
NDB = DL // 128   # d-out blocks (head pairs)
NQ = S // 512     # q blocks
NBS = S // 128    # s tiles / key blocks

_NC = {}


def _build_nc():
    import concourse.bass as bass
    import concourse.mybir as mybir
    import concourse.tile as tile
    from concourse import bacc

    F32 = mybir.dt.float32
    F32R = mybir.dt.float32r
    BF16 = mybir.dt.bfloat16
    Exp = mybir.ActivationFunctionType.Exp

    nc = bacc.Bacc(None)

    xq = nc.dram_tensor("xq", [128, NBK, S], BF16, kind="ExternalInput")
    xk = nc.dram_tensor("xk", [128, NBK, S], BF16, kind="ExternalInput")
    xv = nc.dram_tensor("xv", [128, NBK, S], BF16, kind="ExternalInput")
    wq = nc.dram_tensor("wq", [128, NBK, DL], BF16, kind="ExternalInput")
    wk = nc.dram_tensor("wk", [128, NBK, DL], BF16, kind="ExternalInput")
    wv = nc.dram_tensor("wv", [128, NBK, DL], BF16, kind="ExternalInput")
    wo = nc.dram_tensor("wo", [128, NDB, D], BF16, kind="ExternalInput")
    bqs = nc.dram_tensor("bqs", [128, NDB], F32, kind="ExternalInput")
    bks = nc.dram_tensor("bks", [128, NDB], F32, kind="ExternalInput")
    bvb = nc.dram_tensor("bvb", [128, DL], BF16, kind="ExternalInput")
    bob = nc.dram_tensor("bob", [128, D], BF16, kind="ExternalInput")
    trib = nc.dram_tensor("trib", [128, 2, 128], BF16, kind="ExternalInput")
    onesr = nc.dram_tensor("onesr", [1, DK], F32R, kind="ExternalInput")
    out_d = nc.dram_tensor("out", [S, D], F32, kind="ExternalOutput")

    with tile.TileContext(nc) as tc, nc.allow_low_precision(
            reason="bf16 matmul operands are intended"):
        with (
            tc.tile_pool(name="const", bufs=1) as cpool,
            tc.tile_pool(name="res", bufs=1) as rpool,
            tc.tile_pool(name="xt", bufs=3) as xpool,
            tc.tile_pool(name="pt", bufs=18) as ptpool,
            tc.tile_pool(name="rc", bufs=2) as rcpool,
            tc.tile_pool(name="rb", bufs=2) as rbpool,
            tc.tile_pool(name="ot", bufs=3) as otpool,
            tc.tile_pool(name="mp", bufs=2, space="PSUM") as mpool,
            tc.tile_pool(name="bp", bufs=3, space="PSUM") as bpool,
        ):
            # PE warmup: the HAM clock gate needs ~3.4us of sustained matmul
            # activity to lift the PE clock from 1.2 to 2.4 GHz.  Spend the
            # initial DMA wait running matmuls on a scratch tile so the real
            # projections start warm.
            warm_sb = cpool.tile([128, 128], BF16, name="warm", tag="warm")
            nc.vector.memset(warm_sb[:], 0.0)
            warm_ps = mpool.tile([128, 512], F32, name="mp", tag="mp")
            for _ in range(40):
                nc.tensor.matmul(warm_ps[:, 0:128], warm_sb[:], warm_sb[:],
                                 start=True, stop=True)

            # Startup order matters: the first projection only needs
            # bqs/bks + wq + the first xq chunk, so issue those DMAs first
            # (split per contraction tile so they spread across the 16 DMA
            # queues) and defer the B/C-phase constants until after the
            # first A chunk is emitted.
            bqs_sb = cpool.tile([128, NDB], F32, name="bqs", tag="bqs")
            bks_sb = cpool.tile([128, NDB], F32, name="bks", tag="bks")
            bvb_sb = cpool.tile([128, DL], BF16, name="bvb", tag="bvb")
            bob_sb = cpool.tile([128, D], BF16, name="bob", tag="bob")
            trib_sb = cpool.tile([128, 2, 128], BF16, name="trib", tag="trib")
            onesr_sb = cpool.tile([1, DK], F32R, name="onesr", tag="onesr")
            nc.sync.dma_start(bqs_sb[:], bqs[:])
            nc.sync.dma_start(bks_sb[:], bks[:])

            wq_sb = rpool.tile([128, NBK, DL], BF16, name="wq", tag="wq")
            wk_sb = rpool.tile([128, NBK, DL], BF16, name="wk", tag="wk")
            wv_sb = rpool.tile([128, NBK, DL], BF16, name="wv", tag="wv")
            wo_sb = rpool.tile([128, NDB, D], BF16, name="wo", tag="wo")

            QTc = [[rpool.tile([128, 512], BF16, name=f"QT{i}_{s}", tag=f"QT{i}_{s}")
                    for s in range(NQ)] for i in range(NDB)]
            KTc = [[rpool.tile([128, 512], BF16, name=f"KT{i}_{s}", tag=f"KT{i}_{s}")
                    for s in range(NQ)] for i in range(NDB)]
            # VT is flat [128, 520] = 8 heads x (64 V dims + a ones column);
            # the AV stationary operand is head h's 65-column window, whose
            # ones column makes PSUM row 64 the softmax denominator for free.
            # (65-col LDWEIGHTS gets no Fast Weight Load, but at 65 columns
            # it costs the same as a 128-col FWL load.)
            VT = [rpool.tile([128, HL * (DK + 1)], BF16,
                             name=f"VT{i}", tag=f"VT{i}")
                  for i in range(NBS)]
            OTc = [[rpool.tile([128, 512], BF16, name=f"OT{i}_{s}", tag=f"OT{i}_{s}")
                    for s in range(NQ)] for i in range(NDB)]

            def phase_a_dma(s, first=False):
                sl = slice(s * 512, (s + 1) * 512)
                xts = []
                for xd, w_sb_, wd in ((xq, wq_sb, wq), (xk, wk_sb, wk),
                                      (xv, wv_sb, wv)):
                    xt = xpool.tile([128, NBK, 512], BF16, name="xt", tag="xt")
                    if first:
                        # Interleave per-k weight and activation pieces so the
                        # k-loop of the first projection can start after the
                        # first pair lands and chase the rest.
                        for k in range(NBK):
                            nc.sync.dma_start(w_sb_[:, k, :], wd[:, k, :])
                            nc.sync.dma_start(xt[:, k, :], xd[:, k, sl])
                    else:
                        # Two pieces per tensor -> lands on more DMA queues.
                        nc.sync.dma_start(xt[:, 0:NBK // 2, :],
                                          xd[:, 0:NBK // 2, sl])
                        nc.sync.dma_start(xt[:, NBK // 2:, :],
                                          xd[:, NBK // 2:, sl])
                    xts.append(xt)
                if first:
                    nc.sync.dma_start(bvb_sb[:], bvb[:])
                return xts

            def proj_db(xts, s, which, db):
                xt = xts[which]
                w_sb_ = (wq_sb, wk_sb)[which]
                b_sb = (bqs_sb, bks_sb)[which]
                dst = (QTc, KTc)[which]
                ps = mpool.tile([128, 512], F32, name="mp", tag="mp")
                for k in range(NBK):
                    nc.tensor.matmul(
                        ps[:], w_sb_[:, k, db * 128:(db + 1) * 128],
                        xt[:, k, :], start=(k == 0), stop=(k == NBK - 1))
                nc.vector.tensor_scalar_add(
                    dst[db][s][:], ps[:], b_sb[:, db:db + 1])

            def proj_v(xts, s, mi):
                m = 4 * s + mi
                ps = mpool.tile([128, 512], F32, name="mp", tag="mp")
                for k in range(NBK):
                    nc.tensor.matmul(
                        ps[:], xts[2][:, k, mi * 128:(mi + 1) * 128],
                        wv_sb[:, k, :], start=(k == 0), stop=(k == NBK - 1))
                vt3 = VT[m][:, 0:HL * (DK + 1)].rearrange(
                    "p (h c) -> p h c", c=DK + 1)
                nc.vector.memset(vt3[:, :, DK:DK + 1], 1.0)
                nc.vector.tensor_add(
                    vt3[:, :, 0:DK],
                    ps[:].rearrange("p (h c) -> p h c", c=DK),
                    bvb_sb[:].rearrange("p (h c) -> p h c", c=DK))

            def phase_a_compute(s, xts, skip_v=False):
                # Head-pair 0 of Q/K and all of V before head-pairs 1..3, so
                # B(qb=s, hp=0) unblocks early.
                proj_db(xts, s, 0, 0)
                proj_db(xts, s, 1, 0)
                if not skip_v:
                    phase_a_v(s, xts)
                for db in range(1, NDB):
                    proj_db(xts, s, 0, db)
                    proj_db(xts, s, 1, db)

            def phase_a_v(s, xts):
                for mi in range(4):
                    proj_v(xts, s, mi)

            # Filler queue: next-chunk projection groups get sprinkled into
            # the B phase emission (one 8-matmul group every other key block
            # for the later head pairs), so the exp-paced stretches of the PE
            # FIFO always contain ready work ahead of a waiting score pair.
            filler = {"units": []}

            def filler_load(s, xts):
                u = filler["units"]
                u.append(lambda: proj_db(xts, s, 0, 0))
                u.append(lambda: proj_db(xts, s, 1, 0))
                for mi in range(4):
                    u.append(lambda mi=mi: proj_v(xts, s, mi))
                for db in range(1, NDB):
                    u.append(lambda db=db: proj_db(xts, s, 0, db))
                    u.append(lambda db=db: proj_db(xts, s, 1, db))

            def pump_filler(n=1):
                while n > 0 and filler["units"]:
                    filler["units"].pop(0)()
                    n -= 1

            def flush_filler():
                pump_filler(1 << 30)

            # B runs as a software pipeline over (qb, hp) iterations: while
            # iteration i computes scores+exp (ACT-bound), the AV chain of
            # iteration i-1 -- whose pt tiles are all ready -- streams
            # back-to-back on the PE with no dependency waits.
            bstate = {"pending": [], "norm2": None, "norm2_age": 0}

            # The norm is split in two: stage 1 (DVE reciprocal of the PSUM
            # denominator row + GpSimd broadcast across the 64 d partitions)
            # fires as soon as an AV chain completes; stage 2 (the DVE
            # multiplies that read pso and free its PSUM slot) is deferred a
            # couple of key blocks so the GpSimd broadcast latency is hidden.
            def emit_norm1(it):
                # reciprocal_approx_fast is a bitwise custom DVE op -- it
                # only reads correctly from SBUF, so stage the PSUM
                # denominator row through an SBUF copy first.
                den = rcpool.tile([1, 2, 512], F32, name="rc", tag="rc")
                rden = rcpool.tile([1, 2, 512], F32, name="rc2", tag="rc2")
                rb = rbpool.tile([DK, 2, 512], F32, name="rb", tag="rb")
                # Per-h2 split so the first multiply (which frees the pso
                # PSUM slot) starts one broadcast earlier.
                for h2 in range(2):
                    nc.vector.tensor_copy(
                        den[:, h2, :], it["pso"][DK:DK + 1, h2, :])
                    nc.vector.reciprocal_approx_fast(
                        out=rden[:, h2, :], in_=den[:, h2, :])
                    nc.gpsimd.partition_broadcast(
                        rb[:, h2, :], rden[0:1, h2, :], channels=DK)
                it["rb"] = rb

            def emit_norm1_fast(it):
                # Tail fast path (last iteration): the GpSimd broadcast's
                # ~1.8us latency would sit serially in front of phase_c(3),
                # so broadcast on the (by then idle) PE instead, like the
                # denominator row itself: stationary ones [1, DK], moving
                # f32r denominator row.
                den = rcpool.tile([1, 2, 512], F32R, name="rcf", tag="rcf")
                nc.vector.tensor_copy(den[:], it["pso"][DK:DK + 1, :, :])
                rb = rbpool.tile([DK, 2, 512], F32, name="rbf", tag="rbf")
                for h2 in range(2):
                    psb = mpool.tile([DK, 512], F32, name="mp", tag="mp")
                    nc.tensor.matmul(psb[:], onesr_sb[0:1, :],
                                     den[0:1, h2, :], start=True, stop=True,
                                     skip_group_check=True)
                    nc.vector.reciprocal_approx_fast(
                        out=rb[:, h2, :], in_=psb[:])
                it["rb"] = rb

            def emit_norm2(it, split=False):
                hp, qb, pso, rb = it["hp"], it["qb"], it["pso"], it["rb"]
                if split:
                    # Column-split so phase_c's mi blocks unblock ASAP.
                    for q4 in range(4):
                        ql = slice(q4 * 128, (q4 + 1) * 128)
                        for h2 in range(2):
                            nc.vector.tensor_mul(
                                OTc[hp][qb][h2 * DK:(h2 + 1) * DK, ql],
                                pso[0:DK, h2, ql], rb[:, h2, ql])
                    return
                for h2 in range(2):
                    nc.vector.tensor_mul(
                        OTc[hp][qb][h2 * DK:(h2 + 1) * DK, :],
                        pso[0:DK, h2, :], rb[:, h2, :])

            def flush_norm2():
                it = bstate.get("norm2")
                if it is not None:
                    emit_norm2(it)
                    bstate["norm2"] = None

            def emit_avs(it, n):
                """Emit AV matmuls for iteration `it` up to index n; when the
                chain completes, normalize immediately so the pso slot frees
                as early as possible."""
                if it is None or it.get("done"):
                    return
                while it["emitted"] < min(n, it["kbmax"]):
                    kb, pt_, minq = it["pts"][it["emitted"]]
                    if it["pso"] is None:
                        it["pso"] = bpool.tile([128, 2, 512], F32,
                                               name="bp", tag="bp")
                    for h2 in range(2):
                        # Heads 0..6 use a 128-col stationary window (reaches
                        # into the next head's columns; PSUM rows 65.. are
                        # never read) to get Fast Weight Load; head 7's
                        # window would run off the tile so it uses 65 cols.
                        h = it["hp"] * 2 + h2
                        wc = 128 if h < HL - 1 else DK + 1
                        nc.tensor.matmul(
                            it["pso"][0:wc, h2, minq:512],
                            VT[kb][:, h * (DK + 1):h * (DK + 1) + wc],
                            pt_[:, h2, minq:512],
                            start=(kb == 0), stop=(kb == it["kbmax"] - 1),
                            skip_group_check=True)
                    it["emitted"] += 1
                if it["emitted"] == it["kbmax"]:
                    it["done"] = True
                    if it.get("fast"):
                        emit_norm1_fast(it)
                        emit_norm2(it, split=True)
                    else:
                        emit_norm1(it)
                        flush_norm2()
                        bstate["norm2"] = it
                        bstate["norm2_age"] = 0

            def b_iter(qb, hp, lag=1, pace_delay=0, last=False, fill=0):
                kbmax = 4 * (qb + 1)
                cur = {"hp": hp, "qb": qb, "kbmax": kbmax,
                       "fast": last or hp == NDB - 1,
                       "pts": [], "pso": None, "emitted": 0}
                for kb in range(kbmax):
                    # Ready AV matmuls go in front of the score matmul: the
                    # score may wait on a PSUM slot (exp pacing) and the PE
                    # queue is strict FIFO.
                    if bstate["pending"]:
                        head = bstate["pending"][0]
                        emit_avs(head, kb + 2 - pace_delay)
                        if head.get("done"):
                            bstate["pending"].pop(0)
                    if fill and kb % fill == 1:
                        pump_filler(1)
                    if bstate.get("norm2") is not None:
                        bstate["norm2_age"] += 1
                        if bstate["norm2_age"] >= 2:
                            flush_norm2()
                    di = kb - 4 * qb
                    minq = 128 * di if di > 0 else 0
                    pss = bpool.tile([128, 2, 512], F32, name="bp", tag="bp")
                    for h2 in range(2):
                        base = h2 * DK
                        nc.tensor.matmul(
                            pss[:, h2, minq:512],
                            KTc[hp][kb // 4][base:base + DK,
                                             (kb % 4) * 128:(kb % 4 + 1) * 128],
                            QTc[hp][qb][base:base + DK, minq:512],
                            start=True, stop=True,
                            skip_group_check=True)
                    pt_ = ptpool.tile([128, 2, 512], BF16, name="pt", tag="pt")
                    nc.scalar.activation(pt_[:, :, minq:512],
                                         pss[:, :, minq:512], Exp)
                    if di >= 0:
                        # Causal mask: zero the upper triangle of the exp'd
                        # diagonal block (both heads in one DVE multiply).
                        nc.vector.tensor_mul(
                            pt_[:, :, minq:minq + 128],
                            pt_[:, :, minq:minq + 128], trib_sb[:])
                    cur["pts"].append((kb, pt_, minq))
                    if last and not bstate["pending"]:
                        emit_avs(cur, kb)  # self-AVs trail exp by one block
                bstate["pending"].append(cur)
                maxlag = 0 if last else lag
                while len(bstate["pending"]) > maxlag:
                    head = bstate["pending"].pop(0)
                    emit_avs(head, 1 << 30)
                if last:
                    flush_norm2()

            def phase_c_unit(qb, mi, n2):
                m = 4 * qb + mi
                msl = slice(m * 128, (m + 1) * 128)
                nsl = slice(n2 * 512, (n2 + 1) * 512)
                ps = mpool.tile([128, 512], F32, name="mp", tag="mp")
                for db in range(NDB):
                    nc.tensor.matmul(
                        ps[:], OTc[db][qb][:, mi * 128:(mi + 1) * 128],
                        wo_sb[:, db, nsl],
                        start=(db == 0), stop=(db == NDB - 1))
                ot = otpool.tile([128, 512], F32, name="ob", tag="ob")
                nc.vector.tensor_add(ot[:], ps[:], bob_sb[:, nsl])
                nc.sync.dma_start(out_d[msl, nsl], ot[:])

            def phase_c(qb):
                for mi in range(4):
                    for n2 in range(2):
                        phase_c_unit(qb, mi, n2)

            def filler_load_c(qb):
                for mi in range(4):
                    for n2 in range(2):
                        filler["units"].append(
                            lambda mi=mi, n2=n2: phase_c_unit(qb, mi, n2))

            xts0 = phase_a_dma(0, first=True)
            for t, dt_ in [(trib_sb, trib), (onesr_sb, onesr)]:
                nc.sync.dma_start(t[:], dt_[:])
            # Chunk 0: V projections wait on the last-priority xv/wv DMAs,
            # so emit them after the first score block -- otherwise they
            # block ready Q/K projections and scores in the PE FIFO.
            phase_a_compute(0, xts0, skip_v=True)
            # qb=0: lag 2 + slow AV pacing -- the V DMAs are still landing,
            # so AV matmuls must not sit in the PE FIFO ahead of ready
            # score matmuls.
            b_iter(0, 0, lag=2, pace_delay=2)
            phase_a_v(0, xts0)
            b_iter(0, 1, lag=2, pace_delay=2)
            xts1 = phase_a_dma(1)
            b_iter(0, 2, lag=2, pace_delay=2)
            # wo/bob aren't needed until phase_c(0); don't let their DMAs
            # compete with the startup-critical x/weight streams.
            nc.sync.dma_start(wo_sb[:], wo[:])
            nc.sync.dma_start(bob_sb[:], bob[:])
            b_iter(0, 3, lag=2, pace_delay=2)
            phase_a_compute(1, xts1)
            b_iter(1, 0)          # drains+norms (0,2),(0,3) -> OTc[*][0] done
            xts2 = phase_a_dma(2)
            filler_load_c(0)
            b_iter(1, 1, fill=2)
            filler_load(2, xts2)
            b_iter(1, 2, fill=2)
            b_iter(1, 3, fill=2)
            flush_filler()        # whatever didn't fit in the sprinkles
            b_iter(2, 0)
            xts3 = phase_a_dma(3)
            filler_load_c(1)
            b_iter(2, 1, fill=2)
            filler_load(3, xts3)
            b_iter(2, 2, fill=2)
            b_iter(2, 3, fill=2)
            flush_filler()
            b_iter(3, 0)
            filler_load_c(2)
            b_iter(3, 1, fill=4)
            b_iter(3, 2, fill=4)
            b_iter(3, 3, last=True, fill=4)
            flush_filler()
            phase_c(3)

    nc.finalize()
    return nc


def _to_pkt(a2d, nt):
    """[nt*128, N] -> [128, nt, N] (partition-major tiling of the first dim)."""
    n = a2d.shape[1]
    return np.ascontiguousarray(
        a2d.reshape(nt, 128, n).transpose(1, 0, 2))


def _make_in_maps(query, value, key, Wq, bq, Wk, bk, Wv, bv, Wo, bo):
    import ml_dtypes

    f32 = np.float32
    bf16 = ml_dtypes.bfloat16
    query = np.asarray(query, f32)
    value = np.asarray(value, f32)
    key = np.asarray(key, f32)
    Wq = np.asarray(Wq, f32); bq = np.asarray(bq, f32)
    Wk = np.asarray(Wk, f32); bk = np.asarray(bk, f32)
    Wv = np.asarray(Wv, f32); bv = np.asarray(bv, f32)
    Wo = np.asarray(Wo, f32); bo = np.asarray(bo, f32)

    p = np.arange(128)[:, None]
    j = np.arange(128)[None, :]
    tri01 = np.where(p > j, 0.0, 1.0).astype(bf16)          # [128, 128]
    trib = np.ascontiguousarray(
        np.broadcast_to(tri01[:, None, :], (128, 2, 128)))  # both heads

    xT = {}
    for nm, x in (("q", query), ("k", key), ("v", value)):
        xT[nm] = [_to_pkt(x[b].T.astype(bf16), NBK) for b in range(4)]

    in_maps = []
    for b in range(4):
        for g in range(2):
            sl = slice(g * DL, (g + 1) * DL)
            bo_loc = bo if g == 0 else np.zeros_like(bo)
            m = {
                "xq": xT["q"][b],
                "xk": xT["k"][b],
                "xv": xT["v"][b],
                "wq": _to_pkt((Wq[:, sl] / 8.0).astype(bf16), NBK),
                "wk": _to_pkt(Wk[:, sl].astype(bf16), NBK),
                "wv": _to_pkt(Wv[:, sl].astype(bf16), NBK),
                "wo": _to_pkt(Wo[sl, :].astype(bf16), NDB),
                "bqs": np.ascontiguousarray((bq[sl] / 8.0).reshape(NDB, 128).T),
                "bks": np.ascontiguousarray(bk[sl].reshape(NDB, 128).T),
                "bvb": np.ascontiguousarray(
                    np.broadcast_to(bv[sl][None, :], (128, DL))).astype(bf16),
                "bob": np.ascontiguousarray(
                    np.broadcast_to(bo_loc[None, :], (128, D))).astype(bf16),
                "trib": trib,
                "onesr": np.ones((1, DK), f32),
            }
            in_maps.append(m)
    return in_maps


def kernel_with_info(inputs, trace=False):
    from concourse.bass_utils import run_bass_kernel_spmd

    if "nc" not in _NC:
        _NC["nc"] = _build_nc()

    in_maps = _make_in_maps(**inputs)
    res = run_bass_kernel_spmd(_NC["nc"], in_maps, core_ids=list(range(8)),
                               trace=trace)
    out = np.empty((4, S, D), np.float32)
    for b in range(4):
        out[b] = res.results[2 * b]["out"] + res.results[2 * b + 1]["out"]
    return out, res


def kernel(**inputs):
    out, _ = kernel_with_info(inputs)
    return out
